# revision 14
# baseline (speedup 1.0000x reference)
# GQA causal attention with RoPE on 8 TRN2 NeuronCores (tensor-parallel over heads).
#
# Reference computation (B=2, S=4096, D=2048, H=16 heads, KVH=4 kv heads, HD=128):
#   q/k/v projections -> RoPE on q,k -> causal GQA attention -> o_proj.
#
# Sharding (per hint): core c owns Q heads {2c, 2c+1} and kv head c//2
# (cores 2j/2j+1 redundantly compute kv head j, which is cheap). Each core
# computes attention for its 2 heads over the full sequence, producing the
# transposed attention context [2*HD, B*S]. An AllToAll exchanges it so each
# core holds all 16 heads for its 1/8 slice of the B*S rows; the core then
# applies o_proj (full wo) for that slice and the host concatenates shards.
#
# Layout notes: all matmuls contract over the partition dim, so activations
# are kept transposed (feature dim on partitions): qT/kT/vT = [HD, S]. The
# input x is transposed on the host (part of sharding prep) to xT = [D, B*S].
# Scores are computed transposed, S_T = [s_kv, s_q], via lhsT=kT, rhs=qT;
# softmax-exp runs on the scalar engine (PSUM->SBUF, fused 1/sqrt(HD) scale),
# the causal diagonal uses a multiplicative 0/1 mask, the softmax denominator
# accumulates via a ones-vector matmul, and attn@V uses lhsT=v (natural
# layout, from vT via PE transposes), rhs=exp(S_T), giving out_T = [HD, s_q]
# in PSUM. Matmul operands are bf16 (1 cyc/row on the PE vs 2 for f32); all
# PSUM accumulation is f32. exp() has no max-subtraction: scores here are
# O(5), so exp is comfortably inside f32/bf16 range and softmax is exact up
# to rounding.

import math
import sys

for _p in ("/opt/trn_rl_repo",):
    if _p not in sys.path:
        sys.path.insert(0, _p)

import numpy as np
import ml_dtypes

B = 2
S = 4096
D = 2048
H = 16
KVH = 4
HD = 128
N_CORES = 8
BS = B * S                  # 8192 flattened rows
SHARD = BS // N_CORES       # 1024 output rows per core
HPC = H // N_CORES          # 2 q heads per core
SCALE = 1.0 / math.sqrt(HD)

SQ = 512                    # q-block (matmul free dim)
KV = 128                    # kv-block (psum partition dim)
DCH = D // 128              # 16 contraction chunks for the projections
NB = S // SQ                # 8 q-blocks per batch
NKV_B = S // KV             # 32 kv-blocks per batch
DIAG = SQ // KV             # 4 kv-blocks per q-block on the causal diagonal

BF16 = ml_dtypes.bfloat16

_CACHE = {}
PHASE_MARKS = []


def _mark(nc, phase):
    try:
        PHASE_MARKS.append((phase, int(nc._state.next_id())))
    except Exception:
        pass


def _build(sim_mode=False):
    import concourse.mybir as mybir
    import concourse.tile as tile
    from concourse import bacc

    dt = mybir.dt
    nc = bacc.Bacc("TRN2", target_bir_lowering=False, debug=False,
                   enable_asserts=True, num_devices=N_CORES)

    # ---- external inputs (per-core shards supplied via in_maps) ----
    xT = nc.dram_tensor("xT", [D, BS], dt.bfloat16, kind="ExternalInput")
    cosT = nc.dram_tensor("cosT", [HD, S], dt.bfloat16, kind="ExternalInput")
    sinTs = nc.dram_tensor("sinTs", [HD, S], dt.bfloat16, kind="ExternalInput")
    wq = nc.dram_tensor("wq", [D, HPC * HD], dt.bfloat16, kind="ExternalInput")
    wk = nc.dram_tensor("wk", [D, HD], dt.bfloat16, kind="ExternalInput")
    wv = nc.dram_tensor("wv", [D, HD], dt.bfloat16, kind="ExternalInput")
    wo = nc.dram_tensor("wo", [D, D], dt.bfloat16, kind="ExternalInput")
    masks = nc.dram_tensor("masks", [128, DIAG * SQ], dt.bfloat16, kind="ExternalInput")
    ident = nc.dram_tensor("ident", [128, 128], dt.bfloat16, kind="ExternalInput")
    onesb = nc.dram_tensor("onesb", [128, 1], dt.bfloat16, kind="ExternalInput")
    onesf = nc.dram_tensor("onesf", [1, 128], dt.float32, kind="ExternalInput")
    onesc = nc.dram_tensor("onesc", [128, 1], dt.float32, kind="ExternalInput")

    out = nc.dram_tensor("out", [SHARD, D], dt.float32, kind="ExternalOutput")

    # ---- internal DRAM for the AllToAll ----
    # chunk-major layout: chunk j holds this core's 2 heads for column-shard j.
    ao_in = nc.dram_tensor("ao_in", [N_CORES, HPC * HD, SHARD], dt.bfloat16)
    # after A2A: chunk j holds core j's 2 heads (= global heads 2j, 2j+1)
    # for THIS core's column shard.
    ao_ex = nc.dram_tensor("ao_ex", [N_CORES, HPC * HD, SHARD], dt.bfloat16)

    with tile.TileContext(nc) as tc:
        with tc.tile_pool(name="persist", bufs=1) as pp:
            wq_sb = pp.tile([128, DCH, HPC * HD], dt.bfloat16, name="wq_sb")
            wk_sb = pp.tile([128, DCH, HD], dt.bfloat16, name="wk_sb")
            wv_sb = pp.tile([128, DCH, HD], dt.bfloat16, name="wv_sb")
            wo_sb = pp.tile([128, DCH, D], dt.bfloat16, name="wo_sb")
            cos_sb = pp.tile([HD, S], dt.bfloat16, name="cos_sb")
            sin_sb = pp.tile([HD, S], dt.bfloat16, name="sin_sb")
            mask_sb = pp.tile([128, DIAG * SQ], dt.bfloat16, name="mask_sb")
            id_sb = pp.tile([128, 128], dt.bfloat16, name="id_sb")
            ob_sb = pp.tile([128, 1], dt.bfloat16, name="ob_sb")
            of_sb = pp.tile([1, 128], dt.float32, name="of_sb")
            oc_sb = pp.tile([128, 1], dt.float32, name="oc_sb")

            nc.sync.dma_start(out=wq_sb[:],
                              in_=wq[:].rearrange("(k p) m -> p k m", p=128))
            nc.sync.dma_start(out=wk_sb[:],
                              in_=wk[:].rearrange("(k p) m -> p k m", p=128))
            nc.sync.dma_start(out=wv_sb[:],
                              in_=wv[:].rearrange("(k p) m -> p k m", p=128))
            nc.sync.dma_start(out=mask_sb[:], in_=masks[:])
            nc.sync.dma_start(out=id_sb[:], in_=ident[:])
            nc.sync.dma_start(out=ob_sb[:], in_=onesb[:])
            nc.sync.dma_start(out=of_sb[:], in_=onesf[:])
            nc.sync.dma_start(out=oc_sb[:], in_=onesc[:])

            with tc.tile_pool(name="qkv", bufs=2) as qkvp:
                qts, kts, vns = {}, {}, {}
                for b in range(B):
                    qt = qkvp.tile([HD, HPC, S], dt.bfloat16, name=f"qt{b}", tag="qt")
                    kt = qkvp.tile([HD, S], dt.bfloat16, name=f"kt{b}", tag="kt")
                    vn = qkvp.tile([128, NKV_B, HD], dt.bfloat16, name=f"vn{b}",
                                   tag="vn")
                    qts[b], kts[b], vns[b] = qt, kt, vn
                    _mark(nc, f"A{b}")
                    # ------- phase A: q/k/v projections + RoPE (batch b) -------
                    with tc.tile_pool(name=f"pa{b}", bufs=2) as pa, \
                         tc.tile_pool(name=f"pax{b}", bufs=2) as pax, \
                         tc.tile_pool(name=f"paps{b}", bufs=2, space="PSUM") as paps, \
                         tc.tile_pool(name=f"papv{b}", bufs=1, space="PSUM") as papv, \
                         tc.tile_pool(name=f"patr{b}", bufs=1, space="PSUM") as patr:
                        for si in range(NB):
                            s0 = b * S + si * SQ          # column into xT
                            l0 = si * SQ                  # column into cos/sin
                            pq0 = paps.tile([128, SQ], dt.float32, name="pq0",
                                            tag="pq0")
                            pq1 = paps.tile([128, SQ], dt.float32, name="pq1",
                                            tag="pq1")
                            pk = paps.tile([128, SQ], dt.float32, name="pk", tag="pk")
                            pv = papv.tile([128, SQ], dt.float32, name="pv", tag="pv")
                            xs = pax.tile([128, DCH, SQ], dt.bfloat16, name="xs",
                                          tag="xs")
                            nc.sync.dma_start(
                                out=xs[:],
                                in_=xT[:, s0:s0 + SQ].rearrange(
                                    "(k p) n -> p k n", p=128))
                            if b == 0 and si == 0:
                                # rope tables: first use is ~13us in; emit after
                                # xs0 so the first matmuls aren't starved
                                nc.sync.dma_start(out=cos_sb[:], in_=cosT[:])
                                nc.sync.dma_start(out=sin_sb[:], in_=sinTs[:])
                            for k in range(DCH):
                                st = (k == 0)
                                sp = (k == DCH - 1)
                                nc.tensor.matmul(pq0[:], lhsT=wq_sb[:, k, 0:128],
                                                 rhs=xs[:, k, :], start=st, stop=sp)
                                nc.tensor.matmul(pq1[:], lhsT=wq_sb[:, k, 128:256],
                                                 rhs=xs[:, k, :], start=st, stop=sp)
                                nc.tensor.matmul(pk[:], lhsT=wk_sb[:, k, :],
                                                 rhs=xs[:, k, :], start=st, stop=sp)
                                nc.tensor.matmul(pv[:], lhsT=wv_sb[:, k, :],
                                                 rhs=xs[:, k, :], start=st, stop=sp)
                            # RoPE: dest = p*cos + rot(p)*sin_signed
                            for ph, dest in ((pq0, qt[:, 0, l0:l0 + SQ]),
                                             (pq1, qt[:, 1, l0:l0 + SQ]),
                                             (pk, kt[:, l0:l0 + SQ])):
                                t1 = pa.tile([128, SQ], dt.float32, name="t1",
                                             tag="t1")
                                t2 = pa.tile([128, SQ], dt.float32, name="t2",
                                             tag="t2")
                                nc.vector.tensor_mul(out=t1[:], in0=ph[:],
                                                     in1=cos_sb[:, l0:l0 + SQ])
                                nc.vector.tensor_mul(out=t2[0:64, :],
                                                     in0=ph[64:128, :],
                                                     in1=sin_sb[0:64, l0:l0 + SQ])
                                nc.vector.tensor_mul(out=t2[64:128, :],
                                                     in0=ph[0:64, :],
                                                     in1=sin_sb[64:128, l0:l0 + SQ])
                                nc.vector.tensor_add(out=dest, in0=t1[:], in1=t2[:])
                            # v: cast to bf16, PE-transpose into natural layout
                            vt = pa.tile([128, SQ], dt.bfloat16, name="vt", tag="vt")
                            nc.vector.tensor_copy(out=vt[:], in_=pv[:])
                            for j in range(SQ // 128):
                                ptr = patr.tile([128, 128], dt.bfloat16, name="ptr",
                                                tag="ptr")
                                nc.tensor.transpose(ptr[:],
                                                    vt[:, j * 128:(j + 1) * 128],
                                                    id_sb[:])
                                nc.vector.tensor_copy(out=vn[:, si * DIAG + j, :],
                                                      in_=ptr[:])

                for b in range(B):
                    qt, kt, vn = qts[b], kts[b], vns[b]
                    _mark(nc, f"B{b}")
                    # ------- phase B: causal attention (batch b, 2 heads) -------
                    with tc.tile_pool(name=f"pb{b}", bufs=3) as pb, \
                         tc.tile_pool(name=f"pbn{b}", bufs=2) as pbn, \
                         tc.tile_pool(name=f"pbsc{b}", bufs=2, space="PSUM") as pbsc, \
                         tc.tile_pool(name=f"pbo{b}", bufs=2, space="PSUM") as pbo, \
                         tc.tile_pool(name=f"pbs{b}", bufs=2, space="PSUM") as pbs:
                        for h in range(HPC):
                            aob = pbn.tile([HD, S], dt.bfloat16, name="aob",
                                           tag="aob", bufs=1)
                            for si in range(NB):
                                nkv = (si + 1) * DIAG     # causal kv-block count
                                po = pbo.tile([HD, SQ], dt.float32, name="po",
                                              tag="po")
                                ps = pbs.tile([1, SQ], dt.float32, name="ps", tag="ps")
                                acc = pb.tile([128, SQ], dt.bfloat16, name="acc",
                                              tag="acc")
                                for j2 in range(nkv // 2):
                                    psc = pbsc.tile([128, 2 * SQ], dt.float32,
                                                    name="psc", tag="psc")
                                    et = pb.tile([128, 2 * SQ], dt.bfloat16,
                                                 name="et", tag="et")
                                    for jj in range(2):
                                        j = j2 * 2 + jj
                                        nc.tensor.matmul(
                                            psc[:, jj * SQ:(jj + 1) * SQ],
                                            lhsT=kt[:, j * KV:(j + 1) * KV],
                                            rhs=qt[:, h, si * SQ:(si + 1) * SQ],
                                            start=True, stop=True)
                                    nc.scalar.activation(
                                        et[:], psc[:],
                                        mybir.ActivationFunctionType.Exp,
                                        scale=SCALE)
                                    # causal masks on the diagonal halves first
                                    for jj in range(2):
                                        dd = j2 * 2 + jj - si * DIAG
                                        if dd >= 0:       # causal diagonal block
                                            nc.vector.tensor_mul(
                                                out=et[:, jj * SQ:(jj + 1) * SQ],
                                                in0=et[:, jj * SQ:(jj + 1) * SQ],
                                                in1=mask_sb[:, dd * SQ:(dd + 1) * SQ])
                                    # softmax denominator accumulates on the DVE
                                    if j2 == 0:
                                        nc.vector.tensor_add(
                                            out=acc[:], in0=et[:, 0:SQ],
                                            in1=et[:, SQ:2 * SQ])
                                    else:
                                        nc.vector.tensor_add(
                                            out=acc[:], in0=acc[:], in1=et[:, 0:SQ])
                                        nc.vector.tensor_add(
                                            out=acc[:], in0=acc[:],
                                            in1=et[:, SQ:2 * SQ])
                                    for jj in range(2):
                                        j = j2 * 2 + jj
                                        nc.tensor.matmul(
                                            po[:], lhsT=vn[:, j, :],
                                            rhs=et[:, jj * SQ:(jj + 1) * SQ],
                                            start=(j == 0), stop=(j == nkv - 1))
                                # normalize: po[:, c] * (1/ps[c]) via PE broadcast
                                nc.tensor.matmul(ps[:], lhsT=ob_sb[:], rhs=acc[:],
                                                 start=True, stop=True)
                                rec = pbn.tile([1, SQ], dt.float32, name="rec",
                                               tag="rec")
                                nc.vector.reciprocal(out=rec[:], in_=ps[:])
                                pbc = pbsc.tile([128, SQ], dt.float32, name="pbc",
                                                tag="psc")
                                nc.tensor.matmul(pbc[:], lhsT=of_sb[:], rhs=rec[:],
                                                 start=True, stop=True)
                                bc = pbn.tile([128, SQ], dt.float32, name="bc",
                                              tag="bc")
                                nc.scalar.copy(out=bc[:], in_=pbc[:])
                                nc.vector.tensor_mul(
                                    out=aob[:, si * SQ:(si + 1) * SQ],
                                    in0=po[:], in1=bc[:])
                            # one DMA per (b, h): S columns = 4 A2A shards
                            g0 = b * S
                            nc.sync.dma_start(
                                out=ao_in[g0 // SHARD:(g0 + S) // SHARD,
                                          h * HD:(h + 1) * HD, :].rearrange(
                                    "a p n -> p a n"),
                                in_=aob[:])

            # o_proj weights: needed only in phase D -- fetch behind attention
            nc.sync.dma_start(out=wo_sb[:],
                              in_=wo[:].rearrange("(k p) m -> p k m", p=128))

            _mark(nc, "C")
            # ------- phase C: AllToAll the transposed context -------
            if not sim_mode:
                nc.gpsimd.collective_compute(
                    "AllToAll", mybir.AluOpType.bypass,
                    replica_groups=[list(range(N_CORES))],
                    ins=[ao_in[:]], outs=[ao_ex[:]])
            else:
                ao_ex = ao_in   # single-core TimelineSim: same DMA pattern

            _mark(nc, "D")
            # ------- phase D: o_proj on this core's row shard -------
            with tc.tile_pool(name="pd", bufs=3) as pd, \
                 tc.tile_pool(name="pdps", bufs=4, space="PSUM") as pdps:
                for si in range(SHARD // 128):
                    lt = pd.tile([128, DCH, 128], dt.bfloat16, name="lt", tag="lt")
                    nc.sync.dma_start(
                        out=lt[:],
                        in_=ao_ex[:, :, si * 128:(si + 1) * 128].rearrange(
                            "a (hp p) n -> p (a hp) n", p=128))
                    for dj in range(D // SQ):
                        pod = pdps.tile([128, SQ], dt.float32, name="pod", tag="pod")
                        for k in range(DCH):
                            nc.tensor.matmul(pod[:], lhsT=lt[:, k, :],
                                             rhs=wo_sb[:, k, dj * SQ:(dj + 1) * SQ],
                                             start=(k == 0), stop=(k == DCH - 1))
                        ot = pd.tile([128, SQ], dt.float32, name="ot", tag="ot")
                        nc.vector.tensor_copy(out=ot[:], in_=pod[:])
                        nc.sync.dma_start(
                            out=out[si * 128:(si + 1) * 128, dj * SQ:(dj + 1) * SQ],
                            in_=ot[:])

    nc.compile()
    return nc


def _host_prep(x, cos, sin, wq, wk, wv, wo):
    x = np.asarray(x, dtype=np.float32)
    cos = np.asarray(cos, dtype=np.float32)
    sin = np.asarray(sin, dtype=np.float32)
    wq = np.asarray(wq, dtype=np.float32)
    wk = np.asarray(wk, dtype=np.float32)
    wv = np.asarray(wv, dtype=np.float32)
    wo = np.asarray(wo, dtype=np.float32)

    xT = np.ascontiguousarray(x.reshape(BS, D).T.astype(BF16))         # [D, BS]
    cosT = np.ascontiguousarray(cos[0].T)                              # [HD, S]
    sinT = np.ascontiguousarray(sin[0].T).copy()
    sinT[:64] = -sinT[:64]                      # fold rotate_half sign into sin

    # causal diagonal masks: mask[d][r, c] = 1 iff query col c >= key (d*128+r)
    cc = np.arange(SQ)[None, :]
    rr = np.arange(128)[:, None]
    mtiles = [(cc >= d * 128 + rr).astype(np.float32) for d in range(DIAG)]
    masks = np.ascontiguousarray(np.concatenate(mtiles, axis=1).astype(BF16))

    ident = np.eye(128, dtype=np.float32).astype(BF16)
    onesb = np.ones((128, 1), dtype=np.float32).astype(BF16)
    onesf = np.ones((1, 128), dtype=np.float32)

    wq_bf = wq.astype(BF16)
    wk_bf = wk.astype(BF16)
    wv_bf = wv.astype(BF16)
    wo_bf = np.ascontiguousarray(wo.astype(BF16))

    in_maps = []
    for c in range(N_CORES):
        kvh = c // 2
        in_maps.append({
            "xT": xT,
            "cosT": cosT.astype(BF16),
            "sinTs": sinT.astype(BF16),
            "wq": np.ascontiguousarray(wq_bf[:, c * HPC * HD:(c + 1) * HPC * HD]),
            "wk": np.ascontiguousarray(wk_bf[:, kvh * HD:(kvh + 1) * HD]),
            "wv": np.ascontiguousarray(wv_bf[:, kvh * HD:(kvh + 1) * HD]),
            "wo": wo_bf,
            "masks": masks,
            "ident": ident,
            "onesb": onesb,
            "onesf": onesf,
            "onesc": np.ones((128, 1), dtype=np.float32),
        })
    return in_maps


def kernel(x, cos, sin, wq, wk, wv, wo):
    from concourse.bass_utils import run_bass_kernel_spmd

    if "nc" not in _CACHE:
        _CACHE["nc"] = _build()
    nc = _CACHE["nc"]

    in_maps = _host_prep(x, cos, sin, wq, wk, wv, wo)
    res = run_bass_kernel_spmd(nc, in_maps, core_ids=list(range(N_CORES)))
    shards = [res.results[c]["out"] for c in range(N_CORES)]
    return np.concatenate(shards, axis=0).reshape(B, S, D)
